# revision 1
# baseline (speedup 1.0000x reference)
"""GCN message-passing kernel for Trainium2, 8 NeuronCores.

Math (reference): 3-layer GCN with symmetric normalization and self-loops,
then dot-product decode over label edge pairs.

Key reformulation: A_hat @ (x @ W) == (A_hat @ x) @ W, so each layer is
  agg = A_hat @ x          (sparse gather + scatter)
  z   = relu(agg @ W + b)
A_hat is the same for all 3 layers. All normalization (dinv[src]*dinv[dst],
self-loop dinv^2) is folded into per-edge values.

Device mapping per core (owns 49 consecutive node blocks of 128):
  - edges partitioned by dst block, sorted+chunked into 128-edge chunks
  - dma_gather pulls x[src] rows into SBUF [128 slots, chunks, 128 feat]
    (int16 indices; >=32768 handled by a second gather with the table
    offset by 32768 rows)
  - per chunk, DVE builds indicator[slot, node] = (iota==dstlocal)*norm
    in one tensor_scalar op
  - PE accumulates psum[feat, node] += gathered[slot, feat].T @ indicator
  - per block: z[node, outf] = relu(aggT.T @ W + b) via two matmuls
    (bias via rank-1 ones x bias matmul into the accumulation group)
  - AllGather assembles the full z for the next layer's gathers
Decode: labels bucketed by (a<32768, b<32768); per bucket dma_gather of
z3[a] and z3[b] rows, DVE multiply + reduce, host inverse-permutes.
"""

import numpy as np

P = 128
HALF = 32768
N_CORES = 8


# ---------------------------------------------------------------- host prep

def _wrap16(flat_idx):
    """dma_gather idx layout: idx i at [i%16, i//16], replicated to 128 rows."""
    t = flat_idx.astype(np.int16).reshape(-1, 16).T  # [16, n/16]
    return np.tile(t, (8, 1))  # [128, n/16]


def prepare_edges(edge_index, n_nodes, bpc):
    """Build per-core gather/indicator streams.

    Returns dict with per-core arrays and uniform per-block chunk counts.
    """
    src = np.asarray(edge_index[0], dtype=np.int64)
    dst = np.asarray(edge_index[1], dtype=np.int64)
    deg = np.bincount(dst, minlength=n_nodes).astype(np.float64) + 1.0
    dinv = 1.0 / np.sqrt(deg)

    # full edge list incl self-loops, with folded normalization values
    loops = np.arange(n_nodes, dtype=np.int64)
    esrc = np.concatenate([src, loops])
    edst = np.concatenate([dst, loops])
    enrm = np.concatenate([dinv[src] * dinv[dst], dinv * dinv]).astype(np.float32)

    blk = edst >> 7          # dst block
    dnl = (edst & 127).astype(np.float32)
    n_blocks = N_CORES * bpc
    low = esrc < HALF

    # order edges by (block, highness) once
    order = np.lexsort((~low, blk))
    esrc, dnl_s, enrm_s, blk_s, low_s = (
        esrc[order], dnl[order], enrm[order], blk[order], low[order])

    # per-block counts of low/high edges, per core
    nlow = np.zeros((n_blocks,), np.int64)
    nhigh = np.zeros((n_blocks,), np.int64)
    cnts = np.bincount(blk_s * 2 + (~low_s).astype(np.int64), minlength=2 * n_blocks)
    nlow = cnts[0::2]
    nhigh = cnts[1::2]

    # uniform chunk counts per local block index (max over cores)
    nlow_2d = nlow.reshape(N_CORES, bpc)
    nhigh_2d = nhigh.reshape(N_CORES, bpc)
    cntl = np.maximum(np.ceil(nlow_2d / P).astype(np.int64).max(axis=0), 0)
    cnth = np.maximum(np.ceil(nhigh_2d / P).astype(np.int64).max(axis=0), 0)
    # every block needs at least one chunk so its PSUM group exists
    zero = (cntl + cnth) == 0
    cntl[zero] = 1

    C = int((cntl + cnth).sum())  # chunks per core
    # per-core slot arrays [C*P]
    gidx = np.zeros((N_CORES, C * P), np.int64)
    gdnl = np.zeros((N_CORES, C * P), np.float32)
    gnrm = np.zeros((N_CORES, C * P), np.float32)

    # block start offsets in the sorted edge array
    blk_starts = np.zeros(n_blocks + 1, np.int64)
    np.cumsum(nlow + nhigh, out=blk_starts[1:])

    for c in range(N_CORES):
        pos = 0
        for i in range(bpc):
            b = c * bpc + i
            s0 = blk_starts[b]
            nl, nh = nlow[b], nhigh[b]
            # low slots
            ncap = int(cntl[i]) * P
            take = min(nl, ncap)
            sl = slice(pos, pos + take)
            gidx[c, sl] = esrc[s0:s0 + take]
            gdnl[c, sl] = dnl_s[s0:s0 + take]
            gnrm[c, sl] = enrm_s[s0:s0 + take]
            pos += ncap
            # high slots (stored index is src - HALF)
            hcap = int(cnth[i]) * P
            takeh = min(nh, hcap)
            sh = slice(pos, pos + takeh)
            gidx[c, sh] = esrc[s0 + nl:s0 + nl + takeh] - HALF
            gdnl[c, sh] = dnl_s[s0 + nl:s0 + nl + takeh]
            gnrm[c, sh] = enrm_s[s0 + nl:s0 + nl + takeh]
            pos += hcap
        assert pos == C * P

    # idx stream wrapped for dma_gather: [128, C*P/16] int16
    eidx = np.stack([_wrap16(gidx[c]) for c in range(N_CORES)])
    # dnl/nrm streams indexed [slot partition, chunk]: slot i of chunk k is
    # gather position k*128+i -> array [c, k*128+i] -> reshape [C,P].T
    ednl = np.ascontiguousarray(gdnl.reshape(N_CORES, C, P).transpose(0, 2, 1))
    enrmt = np.ascontiguousarray(gnrm.reshape(N_CORES, C, P).transpose(0, 2, 1))
    return dict(eidx=eidx, ednl=ednl, enrm=enrmt,
                cntl=cntl.astype(int), cnth=cnth.astype(int), C=C)


def prepare_labels(edge_label_index, n_label):
    """Bucket labels by (a<HALF, b<HALF) per core, pad to 128 multiples.

    Returns per-core idx streams for a and b sides, bucket chunk counts
    (uniform across cores), and the per-core slot->label mapping.
    """
    a = np.asarray(edge_label_index[0], dtype=np.int64)
    b = np.asarray(edge_label_index[1], dtype=np.int64)
    per = n_label // N_CORES
    buckets_per_core = []
    for c in range(N_CORES):
        la = a[c * per:(c + 1) * per]
        lb = b[c * per:(c + 1) * per]
        lab = np.arange(c * per, (c + 1) * per)
        bid = (la >= HALF) * 2 + (lb >= HALF)
        buckets_per_core.append([(la[bid == k], lb[bid == k], lab[bid == k])
                                 for k in range(4)])
    tcnt = [max(int(np.ceil(len(buckets_per_core[c][k][0]) / P))
                for c in range(N_CORES)) for k in range(4)]
    T = sum(tcnt)
    aidx = np.zeros((N_CORES, T * P), np.int64)
    bidx = np.zeros((N_CORES, T * P), np.int64)
    labmap = np.full((N_CORES, T * P), -1, np.int64)
    for c in range(N_CORES):
        pos = 0
        for k in range(4):
            la, lb, lab = buckets_per_core[c][k]
            n = len(la)
            cap = tcnt[k] * P
            aidx[c, pos:pos + n] = la - (HALF if k >= 2 else 0)
            bidx[c, pos:pos + n] = lb - (HALF if k % 2 else 0)
            labmap[c, pos:pos + n] = lab
            pos += cap
    la_s = np.stack([_wrap16(aidx[c]) for c in range(N_CORES)])
    lb_s = np.stack([_wrap16(bidx[c]) for c in range(N_CORES)])
    return dict(la=la_s, lb=lb_s, tcnt=tcnt, T=T, labmap=labmap)


# ------------------------------------------------------------- device kernel

def build_bass(n_nodes, bpc, cntl, cnth, tcnt, in_c, hid_c, out_c):
    from concourse import bacc, bass, mybir
    import concourse.tile as tile

    NPAD = N_CORES * bpc * P
    C = int(sum(cntl) + sum(cnth))
    T = int(sum(tcnt))
    f32 = mybir.dt.float32

    nc = bacc.Bacc("TRN2", target_bir_lowering=False, debug=False,
                   num_devices=N_CORES, num_swdge_queues=4)

    x_d = nc.dram_tensor("x", [n_nodes, in_c], f32, kind="ExternalInput")
    w_d = [nc.dram_tensor(f"W{i+1}", s, f32, kind="ExternalInput")
           for i, s in enumerate([[in_c, hid_c], [hid_c, hid_c], [hid_c, out_c]])]
    b_d = [nc.dram_tensor(f"b{i+1}", [s], f32, kind="ExternalInput")
           for i, s in enumerate([hid_c, hid_c, out_c])]
    eidx_d = nc.dram_tensor("eidx", [P, C * P // 16], mybir.dt.int16,
                            kind="ExternalInput")
    ednl_d = nc.dram_tensor("ednl", [P, C], f32, kind="ExternalInput")
    enrm_d = nc.dram_tensor("enrm", [P, C], f32, kind="ExternalInput")
    la_d = nc.dram_tensor("la", [P, T * P // 16], mybir.dt.int16,
                          kind="ExternalInput")
    lb_d = nc.dram_tensor("lb", [P, T * P // 16], mybir.dt.int16,
                          kind="ExternalInput")
    out_d = nc.dram_tensor("out", [P, T], f32, kind="ExternalOutput")

    # internal DRAM: per-layer z slice (local) + allgathered z (shared)
    zs_d = [nc.dram_tensor(f"zs{l}", [bpc * P, w], f32, kind="Internal")
            for l, w in enumerate([hid_c, hid_c, out_c])]
    zf_d = [nc.dram_tensor(f"zf{l}", [NPAD, w], f32, kind="Internal",
                           addr_space="Shared")
            for l, w in enumerate([hid_c, hid_c, out_c])]

    gq = [0]  # round-robin swdge queue

    def next_q():
        q = gq[0]
        gq[0] = (q + 1) % 4
        return q

    with tile.TileContext(nc) as tc:
        with (
            tc.tile_pool(name="consts", bufs=1) as cst,
            tc.tile_pool(name="gath", bufs=6) as gp,
            tc.tile_pool(name="dec", bufs=1) as dp,
            tc.tile_pool(name="work", bufs=8) as wp,
            tc.tile_pool(name="outp", bufs=4) as op,
            tc.tile_pool(name="psum", bufs=4, space="PSUM") as ps,
        ):
            # ---- constants and streams (resident whole kernel)
            iota = cst.tile([P, P], f32)
            nc.gpsimd.iota(iota[:], pattern=[[1, P]], base=0,
                           channel_multiplier=0,
                           allow_small_or_imprecise_dtypes=True)
            ones1 = cst.tile([1, P], f32)
            nc.vector.memset(ones1[:], 1.0)

            eidx_sb = cst.tile([P, C * P // 16], mybir.dt.int16)
            ednl_sb = cst.tile([P, C], f32)
            enrm_sb = cst.tile([P, C], f32)
            nc.sync.dma_start(eidx_sb[:], eidx_d[:, :])
            nc.sync.dma_start(ednl_sb[:], ednl_d[:, :])
            nc.sync.dma_start(enrm_sb[:], enrm_d[:, :])
            la_sb = cst.tile([P, T * P // 16], mybir.dt.int16)
            lb_sb = cst.tile([P, T * P // 16], mybir.dt.int16)
            nc.sync.dma_start(la_sb[:], la_d[:, :])
            nc.sync.dma_start(lb_sb[:], lb_d[:, :])

            w_sb = []
            bias_sb = []
            for l in range(3):
                wt = cst.tile([hid_c if l else in_c, out_c if l == 2 else hid_c], f32)
                nc.sync.dma_start(wt[:], w_d[l][:, :])
                w_sb.append(wt)
                bt = cst.tile([1, out_c if l == 2 else hid_c], f32)
                nc.sync.dma_start(bt[:], b_d[l][None, :])
                bias_sb.append(bt)

            # ---- 3 GCN layers
            for l in range(3):
                oc = out_c if l == 2 else hid_c
                if l == 0:
                    lo_tab = x_d[:, :]
                    hi_tab = x_d[HALF:, :] if n_nodes > HALF else None
                else:
                    zprev = zf_d[l - 1]
                    lo_tab = zprev[:, :]
                    hi_tab = zprev[HALF:, :] if NPAD > HALF else None

                chunk_base = 0
                for i in range(bpc):
                    cl, ch = int(cntl[i]), int(cnth[i])
                    cnt = cl + ch
                    gt = gp.tile([P, cnt * in_c], f32, tag="gath")
                    g3 = gt[:].rearrange("p (c f) -> p c f", c=cnt)
                    if cl:
                        nc.gpsimd.dma_gather(
                            out_ap=g3[:, 0:cl, :] if ch else g3,
                            in_ap=lo_tab,
                            idxs_ap=eidx_sb[:, chunk_base * 8:(chunk_base + cl) * 8],
                            num_idxs=cl * P, num_idxs_reg=cl * P,
                            elem_size=in_c,
                            single_packet=False, queue_num=next_q())
                    if ch:
                        nc.gpsimd.dma_gather(
                            out_ap=g3[:, cl:, :] if cl else g3,
                            in_ap=hi_tab,
                            idxs_ap=eidx_sb[:, (chunk_base + cl) * 8:
                                            (chunk_base + cnt) * 8],
                            num_idxs=ch * P, num_idxs_reg=ch * P,
                            elem_size=in_c,
                            single_packet=False, queue_num=next_q())

                    agg_ps = ps.tile([P, P], f32, tag="agg", space="PSUM")
                    for k in range(cnt):
                        ind = wp.tile([P, P], f32, tag="ind")
                        nc.vector.tensor_scalar(
                            out=ind[:], in0=iota[:],
                            scalar1=ednl_sb[:, chunk_base + k:chunk_base + k + 1],
                            scalar2=enrm_sb[:, chunk_base + k:chunk_base + k + 1],
                            op0=mybir.AluOpType.is_equal,
                            op1=mybir.AluOpType.mult)
                        nc.tensor.matmul(
                            out=agg_ps[:], lhsT=g3[:, k, :], rhs=ind[:],
                            start=(k == 0), stop=(k == cnt - 1))

                    aggT = wp.tile([P, P], f32, tag="aggT")
                    nc.vector.tensor_copy(out=aggT[:], in_=agg_ps[:])

                    z_ps = ps.tile([P, oc], f32, tag="z", space="PSUM")
                    nc.tensor.matmul(out=z_ps[:], lhsT=ones1[:],
                                     rhs=bias_sb[l][:], start=True, stop=False)
                    nc.tensor.matmul(out=z_ps[:], lhsT=aggT[:], rhs=w_sb[l][:],
                                     start=False, stop=True)

                    z_sb = op.tile([P, oc], f32, tag="z_sb")
                    if l < 2:
                        nc.vector.tensor_scalar_max(out=z_sb[:], in0=z_ps[:],
                                                    scalar1=0.0)
                    else:
                        nc.vector.tensor_copy(out=z_sb[:], in_=z_ps[:])
                    nc.sync.dma_start(zs_d[l][i * P:(i + 1) * P, :], z_sb[:])
                    chunk_base += cnt

                nc.gpsimd.collective_compute(
                    "AllGather", mybir.AluOpType.bypass,
                    replica_groups=[list(range(N_CORES))],
                    ins=[zs_d[l][:, :]], outs=[zf_d[l][:, :]])

            # ---- decode
            z3 = zf_d[2]
            tbase = 0
            res = cst.tile([P, T], f32)
            for k in range(4):
                tk = int(tcnt[k])
                if tk == 0:
                    continue
                a_tab = z3[HALF:, :] if (k >= 2 and NPAD > HALF) else z3[:, :]
                b_tab = z3[HALF:, :] if (k % 2 and NPAD > HALF) else z3[:, :]
                ga = dp.tile([P, tk * out_c], f32, tag="ga")
                gb = dp.tile([P, tk * out_c], f32, tag="gb")
                nc.gpsimd.dma_gather(
                    out_ap=ga[:].rearrange("p (c f) -> p c f", c=tk),
                    in_ap=a_tab,
                    idxs_ap=la_sb[:, tbase * 8:(tbase + tk) * 8],
                    num_idxs=tk * P, num_idxs_reg=tk * P, elem_size=out_c,
                    single_packet=False, queue_num=next_q())
                nc.gpsimd.dma_gather(
                    out_ap=gb[:].rearrange("p (c f) -> p c f", c=tk),
                    in_ap=b_tab,
                    idxs_ap=lb_sb[:, tbase * 8:(tbase + tk) * 8],
                    num_idxs=tk * P, num_idxs_reg=tk * P, elem_size=out_c,
                    single_packet=False, queue_num=next_q())
                nc.vector.tensor_mul(out=ga[:], in0=ga[:], in1=gb[:])
                nc.vector.tensor_reduce(
                    out=res[:, tbase:tbase + tk],
                    in_=ga[:].rearrange("p (c f) -> p c f", c=tk),
                    axis=mybir.AxisListType.X, op=mybir.AluOpType.add)
                tbase += tk
            nc.sync.dma_start(out_d[:, :], res[:])

    nc.finalize()
    return nc


# ---------------------------------------------------------------- entry point

def kernel(x, W1, b1, W2, b2, W3, b3, edge_index, edge_label_index):
    from concourse.bass_utils import run_bass_kernel_spmd

    x = np.ascontiguousarray(np.asarray(x, dtype=np.float32))
    n_nodes, in_c = x.shape
    hid_c = np.asarray(W2).shape[0]
    out_c = np.asarray(W3).shape[1]
    n_label = np.asarray(edge_label_index).shape[1]
    bpc = int(np.ceil(n_nodes / (N_CORES * P)))

    ed = prepare_edges(edge_index, n_nodes, bpc)
    lb = prepare_labels(edge_label_index, n_label)

    nc = build_bass(n_nodes, bpc, ed["cntl"], ed["cnth"], lb["tcnt"],
                    in_c, hid_c, out_c)

    common = {
        "x": x,
        "W1": np.ascontiguousarray(np.asarray(W1, np.float32)),
        "W2": np.ascontiguousarray(np.asarray(W2, np.float32)),
        "W3": np.ascontiguousarray(np.asarray(W3, np.float32)),
        "b1": np.ascontiguousarray(np.asarray(b1, np.float32)),
        "b2": np.ascontiguousarray(np.asarray(b2, np.float32)),
        "b3": np.ascontiguousarray(np.asarray(b3, np.float32)),
    }
    in_maps = []
    for c in range(N_CORES):
        m = dict(common)
        m["eidx"] = np.ascontiguousarray(ed["eidx"][c])
        m["ednl"] = np.ascontiguousarray(ed["ednl"][c])
        m["enrm"] = np.ascontiguousarray(ed["enrm"][c])
        m["la"] = np.ascontiguousarray(lb["la"][c])
        m["lb"] = np.ascontiguousarray(lb["lb"][c])
        in_maps.append(m)

    res = run_bass_kernel_spmd(nc, in_maps, core_ids=list(range(N_CORES)))

    out = np.zeros((n_label,), np.float32)
    for c in range(N_CORES):
        o = res.results[c]["out"]  # [P, T]
        flat = o.T.reshape(-1)  # slot i at [i%128, i//128] -> o.T.flat[i]
        lm = lb["labmap"][c]
        valid = lm >= 0
        out[lm[valid]] = flat[valid]
    return out

